# revision 15
# baseline (speedup 1.0000x reference)
"""Phase-Functioned Network (PFNN) forward pass on 8 Trainium2 NeuronCores.

Math: for each sample b, phase selects 4 cyclic weight slices blended by a
Catmull-Rom cubic with parameter mu.  Since NSLICES == 4 and the window is
cyclic, every sample touches all 4 slices, so

    y[b] = sum_s c_s[b] * (W[s] @ h[b] + bias[s])

where c_s[b] is the cubic coefficient that slice s receives for sample b.
This turns the per-sample-weight GEMV into 4 dense GEMMs per layer plus a
per-sample combine.  The combine is folded into the matmuls: a diagonal
matrix D_s = diag(c_s) is multiplied on the PE while transposing activations
(X_s = (c_s * h)^T), after which all 4 slices accumulate into a single PSUM
tile.  The bias term is one extra K=4 matmul with lhsT = c^T.

Sharding: pure data parallel, batch 1024 -> 128 per core; weight slices
(packed/transposed bf16) replicated to all 8 cores.
"""

import numpy as np
import ml_dtypes

import concourse.bacc as bacc
import concourse.mybir as mybir
import concourse.tile as tile
from concourse.bass_utils import run_bass_kernel_spmd
from concourse.vector_clock import ScopedClock


class _FastTileContext(tile.TileContext):
    """TileContext with a minimal kernel tail.

    The stock teardown (2x all-engine barrier + semaphore/DMA range clears)
    costs ~10us and only matters when one loaded NEFF executes twice.  Here
    every kernel() call re-jits a fresh PJRT executable (run_bass_via_pjrt
    builds a new closure), so the NEFF is re-loaded with clean semaphore
    state each run; the final drain (which waits on the full vector clock,
    including output-DMA completion) is all that is needed.
    """

    def _drain_and_barrier(self, tick_clock, wait_clock):
        drain_inst = self.nc.sync.drain()
        wait_clock.add_sem_waits(
            drain_inst.ins, ScopedClock({None: tick_clock.global_clock})
        )
        popped = self.nc._tile_sem_poison_stack.pop()
        assert popped is self._sem_poison

NSLICES = 4
HID = 512
IN_DIM = 342
OUT_DIM = 311
BATCH = 1024
NCORES = 8
BLOC = BATCH // NCORES  # 128
FPAD = 384              # input features zero-padded to 3*128
NWARM = 10              # PE warm-up matmuls (HAM clock-gate release)

F32 = mybir.dt.float32
BF16 = mybir.dt.bfloat16
AL = mybir.AluOpType
ACT = mybir.ActivationFunctionType

# Catmull-Rom basis polynomials a_k(mu) = p3*mu^3 + p2*mu^2 + p1*mu + p0,
# columns k = 0..3 (coefficient rows in Horner order p3, p2, p1, p0).
_CR_POLY = np.array(
    [
        [-0.5, 1.5, -1.5, 0.5],   # mu^3
        [1.0, -2.5, 2.0, -0.5],   # mu^2
        [-0.5, 0.0, 0.5, 0.0],    # mu^1
        [0.0, 1.0, 0.0, 0.0],     # mu^0
    ],
    dtype=np.float32,
)

_CACHE = {}


def _build_program():
    nc = bacc.Bacc("TRN2", num_devices=NCORES, debug=False)

    x_feat = nc.dram_tensor("x_feat", [BLOC, FPAD], F32, kind="ExternalInput")
    phase = nc.dram_tensor("phase", [BLOC, 1], F32, kind="ExternalInput")
    w0 = nc.dram_tensor("w0", [3, 128, NSLICES * HID], BF16, kind="ExternalInput")
    w1 = nc.dram_tensor("w1", [4, 128, NSLICES * HID], BF16, kind="ExternalInput")
    w2 = nc.dram_tensor("w2", [4, 128, NSLICES * OUT_DIM], BF16, kind="ExternalInput")
    b0 = nc.dram_tensor("b0", [NSLICES, HID], BF16, kind="ExternalInput")
    b1 = nc.dram_tensor("b1", [NSLICES, HID], BF16, kind="ExternalInput")
    b2 = nc.dram_tensor("b2", [NSLICES, OUT_DIM], BF16, kind="ExternalInput")
    ident = nc.dram_tensor("ident", [128, 128], BF16, kind="ExternalInput")
    poly = nc.dram_tensor("poly", [BLOC, 4 * NSLICES], F32, kind="ExternalInput")
    y = nc.dram_tensor("y", [BLOC, OUT_DIM], F32, kind="ExternalOutput")

    with _FastTileContext(nc) as tc:
        _kernel_body(nc, tc, x_feat, phase, w0, w1, w2, b0, b1, b2, ident, poly, y)
    nc.compile()
    return nc


def _kernel_body(nc, tc, x_feat, phase, w0, w1, w2, b0, b1, b2, ident, poly, y):
    with (
        tc.tile_pool(name="const", bufs=1) as cpool,
        tc.tile_pool(name="wts", bufs=1) as wpool,
        tc.tile_pool(name="scratch", bufs=2) as apool,
        tc.tile_pool(name="xch", bufs=3) as xpool,
        tc.tile_pool(name="ps_prep", bufs=3, space="PSUM") as ppool,
        tc.tile_pool(name="ps_main", bufs=2, space="PSUM") as mpool,
    ):
        # ---- loads (all HWDGE/sync queue, in consumption order) ----
        i_sb = cpool.tile([128, 128], BF16, tag="ident", name="ident")
        nc.sync.dma_start(out=i_sb[:, :], in_=ident.ap())
        ph = cpool.tile([BLOC, 1], F32, tag="ph", name="ph")
        nc.sync.dma_start(out=ph[:, :], in_=phase.ap())
        p_sb = cpool.tile([BLOC, 4 * NSLICES], F32, tag="poly", name="poly")
        nc.sync.dma_start(out=p_sb[:, :], in_=poly.ap())
        x_f32 = cpool.tile([BLOC, FPAD], F32, tag="x_f32", name="x_f32")
        nc.sync.dma_start(out=x_f32[:, :], in_=x_feat.ap())

        # ---- PE warm-up bridge: keep the HAM clock-gate busy from the first
        # moment the PE can issue until the real matmul stream is ready
        # (~3.4us).  fp32 matmuls are 4 cycles/row = long fillers from few
        # instructions; bf16 ones give fine granularity at the seams.
        warm_ps = ppool.tile([128, HID], F32, tag="warm_ps", name="warm_ps", bufs=1)
        for _ in range(4):
            nc.tensor.matmul(
                warm_ps[:, :128], i_sb[:, :], i_sb[:, :], start=True, stop=True
            )
        for _ in range(2):
            nc.tensor.matmul(
                warm_ps[:, :FPAD], x_f32[:, :128], x_f32[:, :], start=True, stop=True
            )
        for _ in range(4):
            nc.tensor.matmul(
                warm_ps[:, :128], i_sb[:, :], i_sb[:, :], start=True, stop=True
            )

        w0_sb = []
        for j in range(3):
            t = wpool.tile([128, NSLICES * HID], BF16, tag=f"w0_{j}", name=f"w0_{j}")
            nc.sync.dma_start(out=t[:, :], in_=w0.ap()[j])
            w0_sb.append(t)
        b0_sb = wpool.tile([NSLICES, HID], BF16, tag="b0", name="b0")
        nc.sync.dma_start(out=b0_sb[:, :], in_=b0.ap())
        w1_sb = []
        for j in range(4):
            t = wpool.tile([128, NSLICES * HID], BF16, tag=f"w1_{j}", name=f"w1_{j}")
            nc.sync.dma_start(out=t[:, :], in_=w1.ap()[j])
            w1_sb.append(t)
        b1_sb = wpool.tile([NSLICES, HID], BF16, tag="b1", name="b1")
        nc.sync.dma_start(out=b1_sb[:, :], in_=b1.ap())
        w2_sb = []
        for j in range(4):
            t = wpool.tile([128, NSLICES * OUT_DIM], BF16, tag=f"w2_{j}", name=f"w2_{j}")
            nc.sync.dma_start(out=t[:, :], in_=w2.ap()[j])
            w2_sb.append(t)
        b2_sb = wpool.tile([NSLICES, OUT_DIM], BF16, tag="b2", name="b2")
        nc.sync.dma_start(out=b2_sb[:, :], in_=b2.ap())

        # x cast fp32 -> bf16 for the PE (gpsimd — otherwise idle)
        x_sb = cpool.tile([BLOC, FPAD], BF16, tag="x_sb", name="x_sb")
        nc.gpsimd.tensor_copy(out=x_sb[:, :], in_=x_f32[:, :])

        # ---- cubic coefficients c_s[b] (compact chain) ----
        def small(tag, cols=1, dt=F32):
            return cpool.tile([BLOC, cols], dt, tag=tag, name=tag)

        ps = small("ps")
        nc.vector.tensor_scalar(ps[:, :], ph[:, :], 4.0, None, AL.mult)
        # i1 = floor(ps), ps in [0,4): comparison cascade (exact)
        i1 = small("i1")
        nc.vector.tensor_scalar(i1[:, :], ps[:, :], 1.0, None, AL.is_ge)
        nc.vector.scalar_tensor_tensor(i1[:, :], ps[:, :], 2.0, i1[:, :], AL.is_ge, AL.add)
        nc.vector.scalar_tensor_tensor(i1[:, :], ps[:, :], 3.0, i1[:, :], AL.is_ge, AL.add)
        mu = small("mu")
        nc.vector.tensor_tensor(mu[:, :], ps[:, :], i1[:, :], AL.subtract)

        # A[b, k] = a_k(mu[b]) via Horner on host-provided coefficient rows.
        # p_sb columns: [0:4] = p3 row, [4:8] = p2, [8:12] = p1, [12:16] = p0
        aa = small("aa", NSLICES)
        nc.vector.scalar_tensor_tensor(
            aa[:, :], p_sb[:, 0:4], mu[:, :1], p_sb[:, 4:8], AL.mult, AL.add
        )
        nc.vector.scalar_tensor_tensor(
            aa[:, :], aa[:, :], mu[:, :1], p_sb[:, 8:12], AL.mult, AL.add
        )
        nc.vector.scalar_tensor_tensor(
            aa[:, :], aa[:, :], mu[:, :1], p_sb[:, 12:16], AL.mult, AL.add
        )
        # duplicate: a2[b, :] = [A | A] so rotations are contiguous slices
        a2 = small("a2", 2 * NSLICES)
        nc.vector.tensor_copy(out=a2[:, 0:4], in_=aa[:, :])
        nc.vector.tensor_copy(out=a2[:, 4:8], in_=aa[:, :])

        # c_all[b, s] = A[b, (s + 1 - i1[b]) mod 4] = a2[b, (1-i1)%4 + s]
        c_all = small("c_all", NSLICES)
        msk = small("msk")
        for v in range(NSLICES):
            off = (1 - v) % NSLICES
            nc.vector.tensor_scalar(msk[:, :], i1[:, :], float(v), None, AL.is_equal)
            if v == 0:
                nc.vector.tensor_scalar(
                    c_all[:, :], a2[:, off : off + 4], msk[:, :1], None, AL.mult
                )
            else:
                nc.vector.scalar_tensor_tensor(
                    c_all[:, :], a2[:, off : off + 4], msk[:, :1], c_all[:, :],
                    AL.mult, AL.add,
                )

        # D_all[:, s*128:(s+1)*128] = diag(c_s)  (bf16), split across ACT/DVE
        d_all = cpool.tile([128, NSLICES * 128], BF16, tag="d_all", name="d_all")
        for s in range(NSLICES):
            dst = d_all[:, s * 128 : (s + 1) * 128]
            if s % 2 == 0:
                nc.scalar.activation(
                    out=dst, in_=i_sb[:, :], func=ACT.Copy,
                    scale=c_all[:, s : s + 1],
                )
            else:
                nc.vector.tensor_scalar(
                    dst, i_sb[:, :], c_all[:, s : s + 1], None, AL.mult
                )

        # c_rows = c_all^T [4, 128] (for the bias matmul), via PE
        c_bf = small("c_bf", NSLICES, BF16)
        nc.scalar.copy(out=c_bf[:, :], in_=c_all[:, :])
        cr_ps = ppool.tile([NSLICES, 128], F32, tag="cr_ps", name="cr_ps", bufs=1)
        nc.tensor.matmul(cr_ps[:, :], c_bf[:, :], i_sb[:, :], start=True, stop=True)
        c_rows = cpool.tile([NSLICES, 128], BF16, tag="c_rows", name="c_rows")
        nc.scalar.copy(out=c_rows[:, :], in_=cr_ps[:, :])

        # ---- layers ----
        def layer(h_sb, nchunks, w_list, b_tile, n_out):
            """h_sb: [128, nchunks*128] bf16 batch-major activations."""
            psum = mpool.tile([BLOC, n_out], F32, tag="main", name="main")
            first = True
            for j in range(nchunks):
                pp = ppool.tile([128, NSLICES * BLOC], F32, tag="prep", name="prep")
                nc.tensor.matmul(
                    pp[:, :], h_sb[:, j * 128 : (j + 1) * 128], d_all[:, :],
                    start=True, stop=True,
                )
                xj = xpool.tile([128, NSLICES * BLOC], BF16, tag="xch", name="xch")
                if j % 2 == 0:
                    nc.scalar.copy(out=xj[:, :], in_=pp[:, :])
                else:
                    nc.vector.tensor_copy(out=xj[:, :], in_=pp[:, :])
                for s in range(NSLICES):
                    nc.tensor.matmul(
                        psum[:, :],
                        xj[:, s * BLOC : (s + 1) * BLOC],
                        w_list[j][:, s * n_out : (s + 1) * n_out],
                        start=first, stop=False,
                    )
                    first = False
            nc.tensor.matmul(
                psum[:, :], c_rows[:, :], b_tile[:, :], start=False, stop=True
            )
            return psum

        def elu(psum, n, tag):
            # bf16 PE fillers ride through the ELU bubble (results unused)
            for _ in range(5):
                nc.tensor.matmul(
                    warm_ps[:, :], i_sb[:, :], d_all[:, :], start=True, stop=True
                )
            # two independent half-width chains so the next layer's first
            # prep matmuls start after only half the ELU latency
            r = apool.tile([BLOC, n], F32, tag="elu_r", name="elu_r")
            m = apool.tile([BLOC, n], F32, tag="elu_m", name="elu_m")
            e = apool.tile([BLOC, n], F32, tag="elu_e", name="elu_e")
            h = cpool.tile([BLOC, n], BF16, tag=tag, name=tag)
            half = n // 2
            for hs in (slice(0, half), slice(half, n)):
                nc.scalar.activation(out=r[:, hs], in_=psum[:, hs], func=ACT.Relu)
                nc.vector.tensor_scalar(m[:, hs], psum[:, hs], 0.0, None, AL.min)
                nc.scalar.activation(out=e[:, hs], in_=m[:, hs], func=ACT.Exp)
                nc.vector.scalar_tensor_tensor(
                    h[:, hs], e[:, hs], -1.0, r[:, hs], AL.add, AL.add
                )
            return h

        ps0 = layer(x_sb, 3, w0_sb, b0_sb, HID)
        h1 = elu(ps0, HID, "h1")
        ps1 = layer(h1, 4, w1_sb, b1_sb, HID)
        h2 = elu(ps1, HID, "h2")
        ps2 = layer(h2, 4, w2_sb, b2_sb, OUT_DIM)

        o_sb = apool.tile([BLOC, OUT_DIM], F32, tag="osb", name="osb")
        nc.scalar.copy(out=o_sb[:, :], in_=ps2[:, :])
        nc.sync.dma_start(out=y.ap(), in_=o_sb[:, :])


def _pack_weights(W0, W1, W2, b0, b1, b2):
    bf = ml_dtypes.bfloat16
    w0pad = np.zeros((NSLICES, FPAD, HID), np.float32)
    w0pad[:, :IN_DIM, :] = W0.transpose(0, 2, 1)
    w0p = np.ascontiguousarray(
        w0pad.reshape(NSLICES, 3, 128, HID).transpose(1, 2, 0, 3)
        .reshape(3, 128, NSLICES * HID)
    ).astype(bf)
    w1p = np.ascontiguousarray(
        W1.transpose(0, 2, 1).reshape(NSLICES, 4, 128, HID).transpose(1, 2, 0, 3)
        .reshape(4, 128, NSLICES * HID)
    ).astype(bf)
    w2p = np.ascontiguousarray(
        W2.transpose(0, 2, 1).reshape(NSLICES, 4, 128, OUT_DIM).transpose(1, 2, 0, 3)
        .reshape(4, 128, NSLICES * OUT_DIM)
    ).astype(bf)
    return {
        "w0": w0p, "w1": w1p, "w2": w2p,
        "b0": np.asarray(b0, np.float32).astype(bf),
        "b1": np.asarray(b1, np.float32).astype(bf),
        "b2": np.asarray(b2, np.float32).astype(bf),
        "ident": np.eye(128, dtype=np.float32).astype(bf),
        "poly": np.tile(_CR_POLY.reshape(1, 16), (BLOC, 1)).astype(np.float32),
    }


def kernel(x, W0, W1, W2, b0, b1, b2):
    x = np.asarray(x, np.float32)
    if "nc" not in _CACHE:
        _CACHE["nc"] = _build_program()
    nc = _CACHE["nc"]

    shared = _pack_weights(
        np.asarray(W0, np.float32), np.asarray(W1, np.float32),
        np.asarray(W2, np.float32), b0, b1, b2,
    )
    x_feat = np.zeros((BATCH, FPAD), np.float32)
    x_feat[:, :IN_DIM] = x[:, :IN_DIM]
    phase = np.ascontiguousarray(x[:, IN_DIM : IN_DIM + 1])

    in_maps = []
    for c in range(NCORES):
        sl = slice(c * BLOC, (c + 1) * BLOC)
        m = dict(shared)
        m["x_feat"] = np.ascontiguousarray(x_feat[sl])
        m["phase"] = np.ascontiguousarray(phase[sl])
        in_maps.append(m)

    res = run_bass_kernel_spmd(nc, in_maps, core_ids=list(range(NCORES)))
    out = np.concatenate([res.results[c]["y"] for c in range(NCORES)], axis=0)
    return np.ascontiguousarray(out.astype(np.float32))
